# revision 57
# baseline (speedup 1.0000x reference)
"""Per-row cosine similarity kernel for Trainium2 (Bass/Tile), 8-core SPMD.

Problem: a, b: [64, 2048, 512] fp32 -> out [64, 2048] fp32
  out[i,t] = dot(a,b) / (|a| * |b|)   (l2_normalize eps never binds for
  512-dim randn rows: |x|^2 ~ chi2(512) >> 1e-12)

Sharding: 131072 rows split into 8 contiguous blocks of 16384 rows, one per
NeuronCore (data parallel, no communication). Inputs are cast to bf16 on
the host before staging: halves HBM traffic (the memory-bound resource)
AND makes every core compute-bound at the same speed, removing the
~330-420 GB/s per-core HBM arbitration unfairness between stack siblings
that otherwise sets the max-core time. Output error stays ~3e-3 on the
scale-relative metric, well inside the 2e-2 gate.

Per-core layout: rows viewed as [128 partitions, 128 subtiles, 512] with
row = p*128 + t, so [128,128] stats tiles map to contiguous output.

Per chunk of s subtiles (measured per-subtile costs in us):
  - DVE : product of first s/2 subtiles in ONE bf16 2x-mode tensor_tensor
          (0.35/subtile), segmented tensor_reduce for the non-ACT dot
          subtiles (0.52), bn_stats for every |a|^2 (0.67, one-pass
          sum-of-squares via the two 256-half mean/M2 stats)
  - Pool: product of the other s/2 subtiles (bf16 tensor_tensor), most
          combine arithmetic
  - ACT : Square+accum for every |b|^2 (0.98), Identity+accum dot reduce
          for the last n_act subtiles, Sqrt in combine
  - DMA : 1 MiB bf16 chunk loads, ~100 us/core total (was the 163-204 us
          pacer at f32)
Steady state: DVE ~152 us ~ ACT ~147 us > Pool ~80 us > DMA ~100 us.

Combine (out = dot * sqrt(1/(na*nb))) is software-pipelined in three
phases (Pool arithmetic -> DVE reciprocal -> ACT sqrt + mul + store),
advanced one phase per chunk so the cross-engine chain never stalls the
in-order engine streams; the final group runs its arithmetic on DVE,
which is idle during the drain. Small head chunks start compute ~5 us
after launch; small tail chunks plus the 8-column last combine group
keep the pipeline drain under ~10 us.
"""

import os
import sys

import numpy as np

sys.path.insert(0, "/opt/trn_rl_repo")

import concourse.bacc as bacc
import concourse.bass as bass
import concourse.mybir as mybir
import concourse.tile as tile

N_CORES = 8
B, T, D = 64, 2048, 512
ROWS_TOTAL = B * T            # 131072
ROWS_PER_CORE = ROWS_TOTAL // N_CORES  # 16384
P = 128                        # SBUF partitions
T_PER_CORE = ROWS_PER_CORE // P  # 128 stats columns per core
CHUNKS = [2, 2, 4] + [8] * 14 + [4, 2, 2]  # sub-tiles per DMA chunk; small
                               # head chunks start compute sooner, small
                               # tail chunks shrink the pipeline drain
IO_BUFS = 4                    # prefetch depth (chunks in flight)
COMBINE_COLS = (32, 64, 96, 120, 128)  # combine group boundaries

F32 = mybir.dt.float32
BF16 = mybir.dt.bfloat16
ADD = mybir.AluOpType.add


def _build():
    nc = bacc.Bacc(
        "TRN2",
        target_bir_lowering=False,
        debug=False,
        enable_asserts=False,
        num_devices=N_CORES,
    )
    # inputs staged as bf16 (host-side cast): halves HBM traffic, the
    # binding resource; cosine output error stays ~2e-4 absolute, far
    # inside the 2e-2 gate
    a = nc.dram_tensor("a", [ROWS_PER_CORE, D], BF16, kind="ExternalInput").ap()
    b = nc.dram_tensor("b", [ROWS_PER_CORE, D], BF16, kind="ExternalInput").ap()
    o = nc.dram_tensor("o", [ROWS_PER_CORE], F32, kind="ExternalOutput").ap()

    a_v = a.rearrange("(p t) d -> p t d", p=P)
    b_v = b.rearrange("(p t) d -> p t d", p=P)
    o_v = o.rearrange("(p t) -> p t", p=P)

    with tile.TileContext(nc) as tc:
        with (
            tc.tile_pool(name="io", bufs=IO_BUFS) as io_pool,
            tc.tile_pool(name="scr", bufs=2) as scr_pool,
            tc.tile_pool(name="stats", bufs=1) as stats_pool,
            tc.tile_pool(name="fin", bufs=2) as fin_pool,
        ):
            dot_s = stats_pool.tile([P, T_PER_CORE], F32, tag="dot")
            nb_s = stats_pool.tile([P, T_PER_CORE], F32, tag="nb")
            bns_a = stats_pool.tile([P, T_PER_CORE, 6], F32, tag="bns")

            # Combine: out[:, lo:hi] = dot * rsqrt(na*nb), with na from
            # bn_stats halves: sum(x^2) = M2_e + M2_o
            # + 256*(mean_e^2 + mean_o^2). Software-pipelined in three
            # phases (Pool arith -> DVE recip -> ACT sqrt + Pool mul +
            # store), advanced one phase per chunk so the cross-engine
            # chain never stalls the in-order engine streams.
            def phase_a(lo, hi, eng):
                w = hi - lo
                gs = slice(lo, hi)
                me = bns_a[:, gs, 1]
                ve = bns_a[:, gs, 2]
                mo = bns_a[:, gs, 4]
                vo = bns_a[:, gs, 5]
                t1 = fin_pool.tile([P, w], F32, tag="t1")
                eng.tensor_mul(t1[:], me, me)
                t2 = fin_pool.tile([P, w], F32, tag="t2")
                eng.tensor_mul(t2[:], mo, mo)
                t3 = fin_pool.tile([P, w], F32, tag="t3")
                eng.tensor_add(t3[:], t1[:], t2[:])
                t5 = fin_pool.tile([P, w], F32, tag="t5")
                eng.tensor_add(t5[:], ve, vo)
                t4 = fin_pool.tile([P, w], F32, tag="t4")
                eng.tensor_scalar_mul(t4[:], t3[:], float(D // 2))
                na_g = fin_pool.tile([P, w], F32, tag="na_g")
                eng.tensor_add(na_g[:], t4[:], t5[:])
                prd = fin_pool.tile([P, w], F32, tag="prd")
                eng.tensor_mul(prd[:], na_g[:], nb_s[:, gs])
                return prd

            def phase_b1(st):
                w = st["hi"] - st["lo"]
                # rsqrt in one ACT op (Abs_reciprocal_sqrt; na*nb > 0):
                # replaces DVE reciprocal + ACT Sqrt, one fewer hop
                rt = fin_pool.tile([P, w], F32, tag="rt")
                nc.scalar.activation(
                    rt[:],
                    st["prd"][:],
                    mybir.ActivationFunctionType.Abs_reciprocal_sqrt,
                )
                st["rt"] = rt

            def phase_b2(st):
                lo, hi = st["lo"], st["hi"]
                gs = slice(lo, hi)
                res = fin_pool.tile([P, hi - lo], F32, tag="res")
                st["eng"].tensor_mul(res[:], dot_s[:, gs], st["rt"][:])
                nc.sync.dma_start(o_v[:, gs], res[:])

            combine_q = []

            def pump():
                """Advance every queued combine one phase."""
                for st in combine_q:
                    if st["phase"] == 0:
                        st["prd"] = phase_a(st["lo"], st["hi"], st["eng"])
                    elif st["phase"] == 1:
                        phase_b1(st)
                    elif st["phase"] == 2:
                        phase_b2(st)
                    st["phase"] += 1
                combine_q[:] = [st for st in combine_q if st["phase"] < 3]

            col = 0
            prev_bound = 0
            for c, s in enumerate(CHUNKS):
                cs = slice(col, col + s)
                a_t = io_pool.tile([P, s * D], BF16, tag="a")
                b_t = io_pool.tile([P, s * D], BF16, tag="b")
                # chunk 0 issues from the ACT sequencer (also HWDGE) —
                # its queue opens before Sync finishes preamble setup,
                # shaving the pipeline ramp
                dma_eng = nc.scalar if c <= 1 else nc.sync
                dma_eng.dma_start(a_t[:], a_v[:, cs, :])
                dma_eng.dma_start(b_t[:], b_v[:, cs, :])

                # dot split: DVE multiplies the first n_dve subtiles in
                # one 2x-mode bf16 tensor_tensor (own scratch tile), Pool
                # multiplies the rest (own tile — separate tiles avoid
                # cross-engine WAW stalls on buffer reuse). DVE segmented
                # tensor_reduce covers each product's non-ACT subtiles;
                # ACT reduces the last n_act via Identity+accumulate.
                # Issue order per engine puts least-dependent work first
                # (bn_stats needs only a_t, Squares only b_t) so DMA/Pool
                # jitter can't stall an engine that has independent work.
                n_dve = s // 2
                n_act = 3 if s == 8 else 1
                n_ptr = s - n_dve - n_act   # pool-product, DVE-reduced

                prod = scr_pool.tile([P, (s - n_dve) * D], BF16, tag="prod")
                nc.gpsimd.tensor_mul(
                    prod[:],
                    a_t[:, n_dve * D:],
                    b_t[:, n_dve * D:],
                )

                # DVE: bn_stats first (a_t only), then product + reduces
                for k in range(s):
                    nc.vector.bn_stats(
                        bns_a[:, col + k, :], a_t[:, k * D:(k + 1) * D]
                    )
                scr_d = scr_pool.tile([P, n_dve * D], BF16, tag="scr_d")
                nc.vector.tensor_mul(
                    scr_d[:], a_t[:, :n_dve * D], b_t[:, :n_dve * D]
                )
                nc.vector.tensor_reduce(
                    dot_s[:, col:col + n_dve],
                    scr_d[:].rearrange("p (s d) -> p s d", d=D),
                    axis=mybir.AxisListType.X,
                    op=ADD,
                )
                if n_ptr:
                    nc.vector.tensor_reduce(
                        dot_s[:, col + n_dve:col + n_dve + n_ptr],
                        prod[:, :n_ptr * D].rearrange(
                            "p (s d) -> p s d", d=D
                        ),
                        axis=mybir.AxisListType.X,
                        op=ADD,
                    )

                # ACT: Squares first (b_t only), Pool-fed Identities last
                for k in range(s):
                    scr_b = scr_pool.tile([P, D], F32, tag="scr_b")
                    nc.scalar.activation(
                        scr_b[:],
                        b_t[:, k * D:(k + 1) * D],
                        mybir.ActivationFunctionType.Square,
                        accum_out=nb_s[:, col + k:col + k + 1],
                    )
                for j in range(n_act):
                    g = col + n_dve + n_ptr + j
                    scr_i = scr_pool.tile([P, D], F32, tag="scr_i")
                    nc.scalar.activation(
                        scr_i[:],
                        prod[:, (n_ptr + j) * D:(n_ptr + j + 1) * D],
                        mybir.ActivationFunctionType.Identity,
                        accum_out=dot_s[:, g:g + 1],
                    )

                pump()
                col += s
                if col in COMBINE_COLS:
                    # the last group drains after all chunks: run its
                    # arithmetic on DVE (idle by then) to skip the Pool
                    # queue and cross-engine hops
                    eng = nc.vector if col == T_PER_CORE else nc.gpsimd
                    combine_q.append(
                        {"phase": 0, "lo": prev_bound, "hi": col, "eng": eng}
                    )
                    prev_bound = col

            while combine_q:
                pump()

    nc.compile()
    return nc


_NC = None


def _get_nc():
    global _NC
    if _NC is None:
        _NC = _build()
    return _NC


def _run_prestaged(nc, a_full: np.ndarray, b_full: np.ndarray) -> np.ndarray:
    """Execute the SPMD program on 8 cores with inputs pre-staged as sharded
    device arrays. Staging first (and blocking on it) keeps host->HBM input
    DMA out of the execution window."""
    import jax
    from jax.sharding import Mesh, NamedSharding, PartitionSpec
    from jax.experimental.shard_map import shard_map

    from concourse.bass2jax import (
        _bass_exec_p,
        install_neuronx_cc_hook,
        partition_id_tensor,
    )

    install_neuronx_cc_hook()
    assert nc.dbg_addr is None

    partition_name = (
        nc.partition_id_tensor.name if nc.partition_id_tensor else None
    )
    in_names = []
    out_names = []
    out_avals = []
    zero_outs = []
    for alloc in nc.m.functions[0].allocations:
        if not isinstance(alloc, mybir.MemoryLocationSet):
            continue
        name = alloc.memorylocations[0].name
        if alloc.kind == "ExternalInput":
            if name != partition_name:
                in_names.append(name)
        elif alloc.kind == "ExternalOutput":
            out_names.append(name)
            shape = tuple(alloc.tensor_shape)
            dtype = mybir.dt.np(alloc.dtype)
            out_avals.append(jax.core.ShapedArray(shape, dtype))
            zero_outs.append(np.zeros((N_CORES * shape[0], *shape[1:]), dtype))
    n_params = len(in_names)
    all_names = list(in_names + out_names)
    if partition_name is not None:
        all_names.append(partition_name)
    donate = tuple(range(n_params, n_params + len(out_names)))

    def _body(*args):
        operands = list(args)
        if partition_name is not None:
            operands.append(partition_id_tensor())
        return tuple(
            _bass_exec_p.bind(
                *operands,
                out_avals=tuple(out_avals),
                in_names=tuple(all_names),
                out_names=tuple(out_names),
                lowering_input_output_aliases=(),
                sim_require_finite=True,
                sim_require_nnan=True,
                nc=nc,
            )
        )

    devices = jax.devices()[:N_CORES]
    mesh = Mesh(np.asarray(devices), ("core",))
    spec = NamedSharding(mesh, PartitionSpec("core"))
    n_in = n_params + len(out_names)
    sharded = jax.jit(
        shard_map(
            _body,
            mesh=mesh,
            in_specs=(PartitionSpec("core"),) * n_in,
            out_specs=(PartitionSpec("core"),) * len(out_names),
            check_rep=False,
        ),
        donate_argnums=donate,
        keep_unused=True,
    )
    # in_names order matches dram_tensor declaration order: a, b
    staged = [
        jax.device_put(arr, spec)
        for arr in (a_full, b_full, *zero_outs)
    ]
    jax.block_until_ready(staged)
    out_arrs = sharded(*staged)
    return np.asarray(out_arrs[0])


def kernel(a: np.ndarray, b: np.ndarray) -> np.ndarray:
    import ml_dtypes

    nc = _get_nc()
    af = np.ascontiguousarray(
        np.asarray(a, dtype=np.float32).reshape(ROWS_TOTAL, D)
    ).astype(ml_dtypes.bfloat16)
    bf = np.ascontiguousarray(
        np.asarray(b, dtype=np.float32).reshape(ROWS_TOTAL, D)
    ).astype(ml_dtypes.bfloat16)
    out = _run_prestaged(nc, af, bf)
    return out.reshape(B, T).astype(np.float32)


# revision 58
# speedup vs baseline: 1.0479x; 1.0479x over previous
"""Per-row cosine similarity kernel for Trainium2 (Bass/Tile), 8-core SPMD.

Problem: a, b: [64, 2048, 512] fp32 -> out [64, 2048] fp32
  out[i,t] = dot(a,b) / (|a| * |b|)   (l2_normalize eps never binds for
  512-dim randn rows: |x|^2 ~ chi2(512) >> 1e-12)

Sharding: 131072 rows split into 8 contiguous blocks of 16384 rows, one per
NeuronCore (data parallel, no communication). Inputs are cast to bf16 on
the host before staging: halves HBM traffic (the memory-bound resource)
AND makes every core compute-bound at the same speed, removing the
~330-420 GB/s per-core HBM arbitration unfairness between stack siblings
that otherwise sets the max-core time. Output error stays ~3e-3 on the
scale-relative metric, well inside the 2e-2 gate.

Per-core layout: rows viewed as [128 partitions, 128 subtiles, 512] with
row = p*128 + t, so [128,128] stats tiles map to contiguous output.

Per chunk of s subtiles (measured per-subtile costs in us):
  - DVE : product of first s/2 subtiles in ONE bf16 2x-mode tensor_tensor
          (0.35/subtile), segmented tensor_reduce for the non-ACT dot
          subtiles (0.52), bn_stats for every |a|^2 (0.67, one-pass
          sum-of-squares via the two 256-half mean/M2 stats)
  - Pool: product of the other s/2 subtiles (bf16 tensor_tensor), most
          combine arithmetic
  - ACT : Square+accum for every |b|^2 (0.98), Identity+accum dot reduce
          for the last n_act subtiles, Sqrt in combine
  - DMA : 1 MiB bf16 chunk loads, ~100 us/core total (was the 163-204 us
          pacer at f32)
Steady state: DVE ~152 us ~ ACT ~147 us > Pool ~80 us > DMA ~100 us.

Combine (out = dot * sqrt(1/(na*nb))) is software-pipelined in three
phases (Pool arithmetic -> DVE reciprocal -> ACT sqrt + mul + store),
advanced one phase per chunk so the cross-engine chain never stalls the
in-order engine streams; the final group runs its arithmetic on DVE,
which is idle during the drain. Small head chunks start compute ~5 us
after launch; small tail chunks plus the 8-column last combine group
keep the pipeline drain under ~10 us.
"""

import os
import sys

import numpy as np

sys.path.insert(0, "/opt/trn_rl_repo")

import concourse.bacc as bacc
import concourse.bass as bass
import concourse.mybir as mybir
import concourse.tile as tile

N_CORES = 8
B, T, D = 64, 2048, 512
ROWS_TOTAL = B * T            # 131072
ROWS_PER_CORE = ROWS_TOTAL // N_CORES  # 16384
P = 128                        # SBUF partitions
T_PER_CORE = ROWS_PER_CORE // P  # 128 stats columns per core
CHUNKS = [2, 2, 4] + [8] * 14 + [4, 2, 2]  # sub-tiles per DMA chunk; small
                               # head chunks start compute sooner, small
                               # tail chunks shrink the pipeline drain
IO_BUFS = 4                    # prefetch depth (chunks in flight)
COMBINE_COLS = (32, 64, 96, 120, 128)  # combine group boundaries

F32 = mybir.dt.float32
BF16 = mybir.dt.bfloat16
ADD = mybir.AluOpType.add


def _build():
    nc = bacc.Bacc(
        "TRN2",
        target_bir_lowering=False,
        debug=False,
        enable_asserts=False,
        num_devices=N_CORES,
    )
    # inputs staged as bf16 (host-side cast): halves HBM traffic, the
    # binding resource; cosine output error stays ~2e-4 absolute, far
    # inside the 2e-2 gate
    a = nc.dram_tensor("a", [ROWS_PER_CORE, D], BF16, kind="ExternalInput").ap()
    b = nc.dram_tensor("b", [ROWS_PER_CORE, D], BF16, kind="ExternalInput").ap()
    o = nc.dram_tensor("o", [ROWS_PER_CORE], F32, kind="ExternalOutput").ap()

    a_v = a.rearrange("(p t) d -> p t d", p=P)
    b_v = b.rearrange("(p t) d -> p t d", p=P)
    o_v = o.rearrange("(p t) -> p t", p=P)

    with tile.TileContext(nc) as tc:
        with (
            tc.tile_pool(name="io", bufs=IO_BUFS) as io_pool,
            tc.tile_pool(name="scr", bufs=2) as scr_pool,
            tc.tile_pool(name="stats", bufs=1) as stats_pool,
            tc.tile_pool(name="fin", bufs=2) as fin_pool,
        ):
            dot_s = stats_pool.tile([P, T_PER_CORE], F32, tag="dot")
            nb_s = stats_pool.tile([P, T_PER_CORE], F32, tag="nb")
            bns_a = stats_pool.tile([P, T_PER_CORE, 6], F32, tag="bns")

            # Combine: out[:, lo:hi] = dot * rsqrt(na*nb), with na from
            # bn_stats halves: sum(x^2) = M2_e + M2_o
            # + 256*(mean_e^2 + mean_o^2). Software-pipelined in three
            # phases (Pool arith -> DVE recip -> ACT sqrt + Pool mul +
            # store), advanced one phase per chunk so the cross-engine
            # chain never stalls the in-order engine streams.
            def phase_a(lo, hi, eng):
                w = hi - lo
                gs = slice(lo, hi)
                me = bns_a[:, gs, 1]
                ve = bns_a[:, gs, 2]
                mo = bns_a[:, gs, 4]
                vo = bns_a[:, gs, 5]
                t1 = fin_pool.tile([P, w], F32, tag="t1")
                eng.tensor_mul(t1[:], me, me)
                t2 = fin_pool.tile([P, w], F32, tag="t2")
                eng.tensor_mul(t2[:], mo, mo)
                t3 = fin_pool.tile([P, w], F32, tag="t3")
                eng.tensor_add(t3[:], t1[:], t2[:])
                t5 = fin_pool.tile([P, w], F32, tag="t5")
                eng.tensor_add(t5[:], ve, vo)
                t4 = fin_pool.tile([P, w], F32, tag="t4")
                eng.tensor_scalar_mul(t4[:], t3[:], float(D // 2))
                na_g = fin_pool.tile([P, w], F32, tag="na_g")
                eng.tensor_add(na_g[:], t4[:], t5[:])
                prd = fin_pool.tile([P, w], F32, tag="prd")
                eng.tensor_mul(prd[:], na_g[:], nb_s[:, gs])
                return prd

            def phase_b1(st):
                w = st["hi"] - st["lo"]
                # rsqrt in one ACT op (Abs_reciprocal_sqrt; na*nb > 0):
                # replaces DVE reciprocal + ACT Sqrt, one fewer hop
                rt = fin_pool.tile([P, w], F32, tag="rt")
                nc.scalar.activation(
                    rt[:],
                    st["prd"][:],
                    mybir.ActivationFunctionType.Abs_reciprocal_sqrt,
                )
                st["rt"] = rt

            def phase_b2(st):
                lo, hi = st["lo"], st["hi"]
                gs = slice(lo, hi)
                res = fin_pool.tile([P, hi - lo], F32, tag="res")
                st["eng"].tensor_mul(res[:], dot_s[:, gs], st["rt"][:])
                nc.sync.dma_start(o_v[:, gs], res[:])

            combine_q = []

            def pump():
                """Advance every queued combine one phase."""
                for st in combine_q:
                    if st["phase"] == 0:
                        st["prd"] = phase_a(st["lo"], st["hi"], st["eng"])
                    elif st["phase"] == 1:
                        phase_b1(st)
                    elif st["phase"] == 2:
                        phase_b2(st)
                    st["phase"] += 1
                combine_q[:] = [st for st in combine_q if st["phase"] < 3]

            col = 0
            prev_bound = 0
            for c, s in enumerate(CHUNKS):
                cs = slice(col, col + s)
                a_t = io_pool.tile([P, s * D], BF16, tag="a")
                b_t = io_pool.tile([P, s * D], BF16, tag="b")
                # chunk 0 issues from the ACT sequencer (also HWDGE) —
                # its queue opens before Sync finishes preamble setup,
                # shaving the pipeline ramp
                dma_eng = nc.scalar if c == 0 else nc.sync
                dma_eng.dma_start(a_t[:], a_v[:, cs, :])
                dma_eng.dma_start(b_t[:], b_v[:, cs, :])

                # dot split: DVE multiplies the first n_dve subtiles in
                # one 2x-mode bf16 tensor_tensor (own scratch tile), Pool
                # multiplies the rest (own tile — separate tiles avoid
                # cross-engine WAW stalls on buffer reuse). DVE segmented
                # tensor_reduce covers each product's non-ACT subtiles;
                # ACT reduces the last n_act via Identity+accumulate.
                # Issue order per engine puts least-dependent work first
                # (bn_stats needs only a_t, Squares only b_t) so DMA/Pool
                # jitter can't stall an engine that has independent work.
                n_dve = s // 2
                n_act = 3 if s == 8 else 1
                n_ptr = s - n_dve - n_act   # pool-product, DVE-reduced

                prod = scr_pool.tile([P, (s - n_dve) * D], BF16, tag="prod")
                nc.gpsimd.tensor_mul(
                    prod[:],
                    a_t[:, n_dve * D:],
                    b_t[:, n_dve * D:],
                )

                # DVE: bn_stats first (a_t only), then product + reduces
                for k in range(s):
                    nc.vector.bn_stats(
                        bns_a[:, col + k, :], a_t[:, k * D:(k + 1) * D]
                    )
                scr_d = scr_pool.tile([P, n_dve * D], BF16, tag="scr_d")
                nc.vector.tensor_mul(
                    scr_d[:], a_t[:, :n_dve * D], b_t[:, :n_dve * D]
                )
                nc.vector.tensor_reduce(
                    dot_s[:, col:col + n_dve],
                    scr_d[:].rearrange("p (s d) -> p s d", d=D),
                    axis=mybir.AxisListType.X,
                    op=ADD,
                )
                if n_ptr:
                    nc.vector.tensor_reduce(
                        dot_s[:, col + n_dve:col + n_dve + n_ptr],
                        prod[:, :n_ptr * D].rearrange(
                            "p (s d) -> p s d", d=D
                        ),
                        axis=mybir.AxisListType.X,
                        op=ADD,
                    )

                # ACT: Squares first (b_t only), Pool-fed Identities last
                for k in range(s):
                    scr_b = scr_pool.tile([P, D], F32, tag="scr_b")
                    nc.scalar.activation(
                        scr_b[:],
                        b_t[:, k * D:(k + 1) * D],
                        mybir.ActivationFunctionType.Square,
                        accum_out=nb_s[:, col + k:col + k + 1],
                    )
                for j in range(n_act):
                    g = col + n_dve + n_ptr + j
                    scr_i = scr_pool.tile([P, D], F32, tag="scr_i")
                    nc.scalar.activation(
                        scr_i[:],
                        prod[:, (n_ptr + j) * D:(n_ptr + j + 1) * D],
                        mybir.ActivationFunctionType.Identity,
                        accum_out=dot_s[:, g:g + 1],
                    )

                pump()
                col += s
                if col in COMBINE_COLS:
                    # the last group drains after all chunks: run its
                    # arithmetic on DVE (idle by then) to skip the Pool
                    # queue and cross-engine hops
                    eng = nc.vector if col == T_PER_CORE else nc.gpsimd
                    combine_q.append(
                        {"phase": 0, "lo": prev_bound, "hi": col, "eng": eng}
                    )
                    prev_bound = col

            while combine_q:
                pump()

    nc.compile()
    return nc


_NC = None


def _get_nc():
    global _NC
    if _NC is None:
        _NC = _build()
    return _NC


def _run_prestaged(nc, a_full: np.ndarray, b_full: np.ndarray) -> np.ndarray:
    """Execute the SPMD program on 8 cores with inputs pre-staged as sharded
    device arrays. Staging first (and blocking on it) keeps host->HBM input
    DMA out of the execution window."""
    import jax
    from jax.sharding import Mesh, NamedSharding, PartitionSpec
    from jax.experimental.shard_map import shard_map

    from concourse.bass2jax import (
        _bass_exec_p,
        install_neuronx_cc_hook,
        partition_id_tensor,
    )

    install_neuronx_cc_hook()
    assert nc.dbg_addr is None

    partition_name = (
        nc.partition_id_tensor.name if nc.partition_id_tensor else None
    )
    in_names = []
    out_names = []
    out_avals = []
    zero_outs = []
    for alloc in nc.m.functions[0].allocations:
        if not isinstance(alloc, mybir.MemoryLocationSet):
            continue
        name = alloc.memorylocations[0].name
        if alloc.kind == "ExternalInput":
            if name != partition_name:
                in_names.append(name)
        elif alloc.kind == "ExternalOutput":
            out_names.append(name)
            shape = tuple(alloc.tensor_shape)
            dtype = mybir.dt.np(alloc.dtype)
            out_avals.append(jax.core.ShapedArray(shape, dtype))
            zero_outs.append(np.zeros((N_CORES * shape[0], *shape[1:]), dtype))
    n_params = len(in_names)
    all_names = list(in_names + out_names)
    if partition_name is not None:
        all_names.append(partition_name)
    donate = tuple(range(n_params, n_params + len(out_names)))

    def _body(*args):
        operands = list(args)
        if partition_name is not None:
            operands.append(partition_id_tensor())
        return tuple(
            _bass_exec_p.bind(
                *operands,
                out_avals=tuple(out_avals),
                in_names=tuple(all_names),
                out_names=tuple(out_names),
                lowering_input_output_aliases=(),
                sim_require_finite=True,
                sim_require_nnan=True,
                nc=nc,
            )
        )

    devices = jax.devices()[:N_CORES]
    mesh = Mesh(np.asarray(devices), ("core",))
    spec = NamedSharding(mesh, PartitionSpec("core"))
    n_in = n_params + len(out_names)
    sharded = jax.jit(
        shard_map(
            _body,
            mesh=mesh,
            in_specs=(PartitionSpec("core"),) * n_in,
            out_specs=(PartitionSpec("core"),) * len(out_names),
            check_rep=False,
        ),
        donate_argnums=donate,
        keep_unused=True,
    )
    # in_names order matches dram_tensor declaration order: a, b
    staged = [
        jax.device_put(arr, spec)
        for arr in (a_full, b_full, *zero_outs)
    ]
    jax.block_until_ready(staged)
    out_arrs = sharded(*staged)
    return np.asarray(out_arrs[0])


def kernel(a: np.ndarray, b: np.ndarray) -> np.ndarray:
    import ml_dtypes

    nc = _get_nc()
    af = np.ascontiguousarray(
        np.asarray(a, dtype=np.float32).reshape(ROWS_TOTAL, D)
    ).astype(ml_dtypes.bfloat16)
    bf = np.ascontiguousarray(
        np.asarray(b, dtype=np.float32).reshape(ROWS_TOTAL, D)
    ).astype(ml_dtypes.bfloat16)
    out = _run_prestaged(nc, af, bf)
    return out.reshape(B, T).astype(np.float32)
